# revision 20
# baseline (speedup 1.0000x reference)
"""Trainium2 Bass kernel for nn_BasePolicyNetwork (Dirichlet policy head).

Reference computation:
    state = concat([bias[:,None], weight], 1)          # [N, 513]
    v     = state @ wv.T                               # [N, 20]  (q,k are dead code)
    alpha = softmax(v + prior, axis=1)                 # Dirichlet concentrations
    g     = jax.random.gamma(key(42), alpha)
    out   = g / g.sum(1, keepdims=True)

Device strategy (pure data parallel over N across 8 NeuronCores):
  - Host transposes/packs weight so every job load is one fully
    contiguous DRAM read; each core streams its 16 MiB fp16 shard on
    the sync HWDGE ring at the ~390 GB/s per-core cap (16 engines x
    ~25.5 GB/s; the scalar ring is ~5x slower for bulk, gpsimd DMA is
    ~15 GB/s software-DGE) and computes v_w.T [20, 16384] on the
    TensorEngine, accumulating the 512-deep contraction across 8 PSUM
    banks. fp16 halves the DMA bytes (the bottleneck); measured final
    rel err 1.56e-3 vs the 2e-2 gate (fp8 fails: the small-alpha gamma
    tail amplifies concentration error ~30x; even 64/512 fp8 columns
    measured 1.66e-2).
  - The whole shard is SBUF-resident (128 KiB/partition): one dedicated
    tile per DMA job, so all job configs pre-issue with no buffer-reuse
    stalls.  Consumers wait for FULL job completion, so jobs taper
    4096 -> 128 rows: big jobs early give 32 KiB descriptors, small
    jobs at the end let the last matmul land ~1us after the last HBM
    byte.
  - PSUM->SBUF f16 casts alternate ACT / DVE (last cast on the faster
    DVE); stores ride the scalar ring mid-run, the penultimate store
    rides the idle sync ring and the final 5 KiB store rides scalar so
    the two tail configs (~700ns each) execute in parallel: last cast
    -> all data in DRAM is ~1.9us.
  - The rank-1 bias channel contribution (bias x wv[:,0]) is folded in
    on the host (0.002% of the FLOPs).
  - The Dirichlet sampling tail (softmax + gamma + normalize) must be
    bit-compatible with the reference's jax.random.gamma rejection
    sampler, so it runs through the exact same jax op sequence with
    threefry keys on CPU jax (see comment in kernel()).
"""

import os
import sys

for _p in ("/opt/trn_rl_repo",):
    if _p not in sys.path and os.path.isdir(_p):
        sys.path.insert(0, _p)

import numpy as np

N_TOTAL = 131072
N_CORES = 8
R = N_TOTAL // N_CORES  # 16384 rows per core
K_W = 512               # weight channels on device
C = 20                  # output channels
BIGF = 2048             # rows per state DMA chunk (2 MiB at fp16)
RT = 512                # rows per matmul / psum tile
NBIG = R // BIGF        # 8
SUBT = BIGF // RT       # 4

_MM_DT_NAME = os.environ.get("KERNEL_MM_DTYPE", "float16")

# Whole fp16 shard (128 KiB/partition) stays resident in SBUF: every DMA
# job gets a dedicated tile, so no job ever waits on buffer reuse and all
# job configs are issued up front on the sync queue.  Matmuls wait for
# FULL job completion, so jobs must stay small enough that the PE trails
# the stream closely (2048 rows = 5.3us stream vs 3.5us matmul), tapering
# to 64 rows at the end so the last matmul lands <1us after the last HBM
# byte (a 4096-row mid-job was measured to push the matmul tail out 6us).
_BLOCKS = [4096] * 3 + [1024] * 3 + [512, 256, 128, 128]
assert sum(_BLOCKS) == R
# rows after which v is flushed to DRAM.  Mid-run stores ride the scalar
# HWDGE ring (configs hide in idle scalar time); the penultimate store
# rides the by-then-idle sync ring and the final 64-row store rides
# scalar, so the two tail configs execute on DIFFERENT queues in
# parallel and the critical tail is one cast + one ~700ns config + 2.5KB.
_STORE_ROWS = [4096, 8192, 12288, 15360, 16256, 16384]
_STORE_ENG = ["scalar", "scalar", "scalar", "scalar", "sync", "scalar"]

_NP_DT = {
    "float32": np.float32,
    "float32r": np.float32,
    "float16": np.float16,
}

_BUILT = {}


def _build():
    if "nc" in _BUILT:
        return _BUILT["nc"]

    import concourse.mybir as mybir
    import concourse.tile as tile
    from concourse import bacc

    mm_dt = getattr(mybir.dt, _MM_DT_NAME)
    f32 = mybir.dt.float32

    nc = bacc.Bacc("TRN2", target_bir_lowering=False, debug=False,
                   num_devices=N_CORES)

    f16 = mybir.dt.float16
    weightT = nc.dram_tensor("weightT", [K_W * R], mm_dt, kind="ExternalInput")
    wvp = nc.dram_tensor("wvp", [128, 4 * C], mm_dt, kind="ExternalInput")
    vout = nc.dram_tensor("vout", [C, R], f16, kind="ExternalOutput")

    blocks = _BLOCKS

    with tile.TileContext(nc) as tc:
        with (
            tc.tile_pool(name="constp", bufs=1) as constp,
            tc.tile_pool(name="statep", bufs=1) as statep,
            tc.tile_pool(name="outp", bufs=1) as outp,
            tc.tile_pool(name="psump", bufs=8, space="PSUM") as psump,
        ):
            wv_sb = constp.tile([128, 4 * C], mm_dt)
            # everything bulk rides the sync HWDGE ring: the scalar ring
            # was measured ~5x slower for large loads (job0 there pushed
            # the first matmul from 15.6us to 27.2us and dragged the sync
            # ring to 319 GB/s)
            nc.sync.dma_start(wv_sb[:], wvp[:])

            out_sb = outp.tile([C, R], f16)

            st_flat = weightT.ap()

            # issue every weight-job config up front (dedicated SBUF tile
            # per job -> no semaphore waits on the configs, the sync ring
            # never idles between jobs)
            st_sbs = []
            off = 0
            for bi, blk in enumerate(blocks):
                st_sb = statep.tile([128, 4, blk], mm_dt, tag=f"st{bi}",
                                    bufs=1, name=f"st_{bi}")
                src = st_flat[off:off + 128 * 4 * blk].rearrange(
                    "(p c n) -> p c n", p=128, c=4
                )
                nc.sync.dma_start(st_sb[:, :, :blk], src)
                off += 128 * 4 * blk
                st_sbs.append(st_sb)

            # PSUM -> SBUF f16 casts alternate ACT / DVE so a single cast
            # chain never paces PSUM-bank reuse against the warm PE; with
            # 34 tiles the LAST cast (odd index) lands on the faster DVE.
            cast_ops = [
                lambda o, i: nc.scalar.copy(o, i),
                nc.vector.tensor_copy,
            ]

            r0 = 0
            ti = 0
            store_r0 = 0
            si = 0
            for bi, blk in enumerate(blocks):
                st_sb = st_sbs[bi]
                for s in range((blk + RT - 1) // RT):
                    w = min(RT, blk - s * RT)
                    rt0 = r0 + s * RT
                    ps = psump.tile([C, RT], f32, tag="ps")
                    for c in range(4):
                        nc.tensor.matmul(
                            ps[:, :w],
                            wv_sb[:, c * C:(c + 1) * C],
                            st_sb[:, c, s * RT:s * RT + w],
                            start=(c == 0),
                            stop=(c == 3),
                        )
                    cast_ops[ti % 2](out_sb[:, rt0:rt0 + w], ps[:, :w])
                    ti += 1
                r0 += blk
                if si < len(_STORE_ROWS) and r0 == _STORE_ROWS[si]:
                    eng = getattr(nc, _STORE_ENG[si])
                    eng.dma_start(
                        vout[:, store_r0:r0], out_sb[:, store_r0:r0]
                    )
                    store_r0 = r0
                    si += 1

    nc.compile()
    _BUILT["nc"] = nc
    return nc


def _run_device(weight_packs, wvp: np.ndarray, trace: bool = False):
    from concourse import bass_utils

    nc = _build()
    in_maps = [{"weightT": weight_packs[i], "wvp": wvp} for i in range(N_CORES)]
    res = bass_utils.run_bass_kernel_spmd(
        nc, in_maps, core_ids=list(range(N_CORES)), trace=trace,
    )
    v = np.empty((N_TOTAL, C), np.float32)
    for i in range(N_CORES):
        v[i * R:(i + 1) * R] = res.results[i]["vout"].T.astype(np.float32)
    return v, res


def _pack_inputs(bias, weight, wv):
    np_dt = _NP_DT[_MM_DT_NAME]
    w16 = weight.astype(np_dt)
    packs = []
    for i in range(N_CORES):
        shard = w16[i * R:(i + 1) * R]
        pack = np.empty(R * K_W, np_dt)
        off = 0
        r0 = 0
        for blk in _BLOCKS:
            seg = pack[off:off + blk * K_W].reshape(128, 4, blk)
            seg[:] = shard[r0:r0 + blk].reshape(blk, 4, 128).transpose(2, 1, 0)
            off += blk * K_W
            r0 += blk
        packs.append(pack)
    wvp = np.empty((128, 4 * C), np_dt)
    for c in range(4):
        wvp[:, c * C:(c + 1) * C] = wv[:, 1 + c * 128: 1 + (c + 1) * 128].T
    return packs, wvp


def kernel(bias, weight, prior, wq, wk, wv, rel_h, rel_w):
    import jax
    import jax.numpy as jnp

    bias = np.asarray(bias, np.float32)
    weight = np.asarray(weight, np.float32)
    prior = np.asarray(prior, np.float32)
    wv = np.asarray(wv, np.float32)

    weightT, wvp = _pack_inputs(bias, weight, wv)
    v, _ = _run_device(weightT, wvp)

    v = v + bias[:, None] * wv[None, :, 0]

    with jax.default_device(jax.devices("cpu")[0]):
        concen = jnp.asarray(v)
        new_concen = jax.nn.softmax(concen + jnp.asarray(prior), axis=1)
        key = jax.random.key(42, impl="threefry2x32")
        g = jax.random.gamma(key, new_concen)
        out = g / jnp.sum(g, axis=1, keepdims=True)
        return np.asarray(out, np.float32)

